# revision 16
# baseline (speedup 1.0000x reference)
"""GroupSortActivation (GROUP_SIZE=2) Trainium2 Bass kernel.

out[:, 2i]   = min(x[:, 2i], x[:, 2i+1])
out[:, 2i+1] = max(x[:, 2i], x[:, 2i+1])

Sharding: batch dim (16384) split evenly across 8 NeuronCores (2048 rows
per core); no communication. Per core: stream 16 tiles of (128, 4096)
fp32 (2MB, one DRAM row per partition = 16KB contiguous per partition),
two strided tensor_tensor ops (min/max) on DVE, stream back out.
Measured ~175us/core on HW = ~366 GB/s/core of the ~436 GB/s fabric cap;
DMA-bound with all 16 SDMA engines ~96% busy.

Raw-bass pipeline (walrus limits attached sync-waits per instruction —
TensorTensor allows only 1 and HWDGE DIRECT2D DMA allows none/one — so
all waits are standalone sequencer instructions):
  SP  (sync):   loads  x -> t[i%NB]  (HWDGE), slot gated on DVE progress
  DVE (vector): waits store-slot free + load done, then min/max
  ACT (scalar): stores o[i%NO] -> y  (HWDGE), gated on DVE progress
Per-slot DMA-completion semaphores make out-of-order DMA completion safe.
fp32 tensor_tensor runs in 1x DVE mode regardless of stride, so the
stride-2 access patterns cost nothing extra; compute (~70us/core) hides
entirely under DMA (~158us/core busy).
"""

import numpy as np

import concourse.bass as bass
from concourse import mybir
from concourse.bass_utils import run_bass_kernel_spmd

N_CORES = 8
B, D = 16384, 4096
RPC = B // N_CORES  # rows per core = 2048
P = 128  # SBUF partitions
N_TILES = RPC // P  # 16 tiles of (128, 4096)
NB = 4  # input slots  (4 x 2MB)
NO = 4  # output slots (4 x 2MB)


def build_nc() -> bass.Bass:
    nc = bass.Bass()
    x = nc.dram_tensor("x", [RPC, D], mybir.dt.float32, kind="ExternalInput")
    y = nc.dram_tensor("y", [RPC, D], mybir.dt.float32, kind="ExternalOutput")

    from contextlib import ExitStack

    with ExitStack() as ctx:
        t = [
            ctx.enter_context(nc.sbuf_tensor(f"t{j}", [P, D], mybir.dt.float32))
            for j in range(NB)
        ]
        o = [
            ctx.enter_context(nc.sbuf_tensor(f"o{k}", [P, D], mybir.dt.float32))
            for k in range(NO)
        ]
        ld = [ctx.enter_context(nc.semaphore(f"ld{j}")) for j in range(NB)]
        st = [ctx.enter_context(nc.semaphore(f"st{k}")) for k in range(NO)]
        dv = ctx.enter_context(nc.semaphore("dv"))
        ld0b = ctx.enter_context(nc.semaphore("ld0b"))

        block = ctx.enter_context(nc.Block())

        H = D // 2  # column half for head/tail pipeline-fill trimming
        LAST = N_TILES - 1
        # DVE ops per tile: first and last tiles are computed in 2 halves
        ops = [4 if i in (0, LAST) else 2 for i in range(N_TILES)]
        cum = [0] * N_TILES
        run = 0
        for i in range(N_TILES):
            run += ops[i]
            cum[i] = run

        @block.sync
        def _(sync):
            for i in range(N_TILES):
                j = i % NB
                if i >= NB:
                    # input slot j free once tile i-NB fully computed
                    sync.wait_ge(dv, cum[i - NB])
                if i == 0:
                    # split first load so compute can start after 1MB
                    sync.dma_start(t[j][:, 0:H], x[0:P, 0:H]).then_inc(ld[j], 16)
                    sync.dma_start(t[j][:, H:D], x[0:P, H:D]).then_inc(ld0b, 16)
                else:
                    sync.dma_start(t[j][:], x[i * P : (i + 1) * P, :]).then_inc(
                        ld[j], 16
                    )

        def halves(vector, j, k, c0, c1):
            te, to = t[j][:, c0:c1:2], t[j][:, c0 + 1 : c1 : 2]
            vector.tensor_tensor(
                o[k][:, c0:c1:2], te, to, op=mybir.AluOpType.min
            ).then_inc(dv, 1)
            vector.tensor_tensor(
                o[k][:, c0 + 1 : c1 : 2], te, to, op=mybir.AluOpType.max
            ).then_inc(dv, 1)

        @block.vector
        def _(vector):
            for i in range(N_TILES):
                j, k = i % NB, i % NO
                if i >= NO:
                    # output slot k free once tile i-NO's store completed
                    vector.wait_ge(st[k], 16 * (i // NO))
                if i == 0:
                    vector.wait_ge(ld[j], 16)
                    halves(vector, j, k, 0, H)
                    vector.wait_ge(ld0b, 16)
                    halves(vector, j, k, H, D)
                elif i == LAST:
                    vector.wait_ge(ld[j], 16 * (i // NB + 1))
                    halves(vector, j, k, 0, H)  # store of first half can begin
                    halves(vector, j, k, H, D)
                else:
                    vector.wait_ge(ld[j], 16 * (i // NB + 1))
                    halves(vector, j, k, 0, D)

        @block.scalar
        def _(scalar):
            for i in range(N_TILES):
                k = i % NO
                if i == LAST:
                    # split last store: first half overlaps second half's compute
                    scalar.wait_ge(dv, cum[i] - 2)
                    scalar.dma_start(y[i * P : (i + 1) * P, 0:H], o[k][:, 0:H]).then_inc(
                        st[k], 16
                    )
                    scalar.wait_ge(dv, cum[i])
                    scalar.dma_start(y[i * P : (i + 1) * P, H:D], o[k][:, H:D]).then_inc(
                        st[k], 16
                    )
                else:
                    scalar.wait_ge(dv, cum[i])
                    scalar.dma_start(y[i * P : (i + 1) * P, :], o[k][:]).then_inc(
                        st[k], 16
                    )
            # make sure every store landed before the program ends
            for k in range(NO):
                incs = sum(2 if i == LAST else 1 for i in range(N_TILES) if i % NO == k)
                scalar.wait_ge(st[k], 16 * incs)

    return nc


_NC_CACHE = None


def _get_nc() -> bass.Bass:
    global _NC_CACHE
    if _NC_CACHE is None:
        _NC_CACHE = build_nc()
    return _NC_CACHE


def make_in_maps(x: np.ndarray) -> list[dict[str, np.ndarray]]:
    xs = np.ascontiguousarray(np.asarray(x), dtype=np.float32)
    assert xs.shape == (B, D), xs.shape
    return [{"x": xs[i * RPC : (i + 1) * RPC]} for i in range(N_CORES)]


def kernel(x: np.ndarray) -> np.ndarray:
    res = run_bass_kernel_spmd(_get_nc(), make_in_maps(x), list(range(N_CORES)))
    return np.concatenate([r["y"] for r in res.results], axis=0)
